# revision 16
# baseline (speedup 1.0000x reference)
"""Trainium2 Bass kernel for the CrossAttention problem (self-contained).

Strategy: shard the N=4096 query rows across 8 cores (512 rows/core, both
batch elements). Everything is computed in transposed layout (features on
partitions, query rows on the free dim) so every matmul has a wide moving
operand.

v2 structure (vs the v1 baseline):
 - A dummy 32-byte AllGather is issued at t=0 so the expensive first-collective
   device barrier (~40us) runs concurrently with input DMA / early compute
   instead of on the wmask critical path.
 - A short burst of junk matmuls at t=0 warms the PE HAM clock gate during the
   initial input-DMA window.
 - Phase A computes only what the collective needs (cond-half qT, branch-0 kT,
   branch-0 QK + row max) off a lean input-DMA set, then posts the real
   AllGather.  Branch-0 sims are immediately exponentiated and parked as f16
   (exp(s + w*ae) == exp(s) * exp(w*ae), so the mask can be applied
   multiplicatively later).
 - Phase B does everything not branch-0-dependent, including the PV
   accumulation of cond branches 1..3 (partials parked in SBUF) and the uc
   half of the output projection.
 - The tail (after the AllGather lands) only has to do the branch-0 softmax
   (mask via exp(w*ae) multiply), add its PV contribution, and run the cond
   half of the output projection.
 - PV matmuls for a head pair run col-tiled (tile_position (0,0)/(0,64)) into
   one PSUM bank; QK matmuls stay row-tiled pairs.
 - Softmax normalization 1/Z per unit is computed either with the DVE
   reciprocal or as exp(-ln Z) on the scalar engine, and the attn multiply on
   DVE or GpSimd, per a static balance schedule (VARIANTS below).
"""

import sys

sys.path.insert(0, "/opt/trn_rl_repo")

import numpy as np

import concourse.bass as bass
import concourse.tile as tile
from concourse import bacc, bass_utils, mybir

# problem constants (hardcoded per the harness contract)
H, DH, L, C = 8, 64, 77, 4
N, DQ, DC, INNER = 4096, 320, 768, 512
N_CORES = 8
NS = N // N_CORES          # query rows per core per batch element
NSB = 2 * NS               # both batch elements
SCALE = DH ** -0.5
W_DOT, TOTAL_STEP, SCHED = 1.0, 50, 4.6

F32 = mybir.dt.float32
F16 = mybir.dt.float16
EXP = mybir.ActivationFunctionType.Exp
LN = mybir.ActivationFunctionType.Ln
AXX = mybir.AxisListType.X

# groups in context order: 0=uc, 1..4 = cond branches 0..3
G_UC = 0

LAST_RESULTS = None  # BassKernelResults of the most recent run (for test.py)
TRACE = False

N_WARMUP_MM = 14

# normalization variant per unit: 'rd' recip + DVE mul, 'rg' recip + gpsimd
# mul, 'ld' lnexp + DVE mul, 'lg' lnexp + gpsimd mul.
# cond units keyed (hp, g) for g in 2,3,4; tail units keyed by hp.
COND_VAR = {
    (0, 2): 'rd', (0, 3): 'rg', (0, 4): 'rg',
    (1, 2): 'lg', (1, 3): 'rd', (1, 4): 'rg',
    (2, 2): 'rg', (2, 3): 'lg', (2, 4): 'rd',
    (3, 2): 'rg', (3, 3): 'rg', (3, 4): 'lg',
}
TAIL_VAR = {0: 'rd', 1: 'lg', 2: 'rd', 3: 'lg'}


def _gate(a, b, reason="order"):
    if a is not None and b is not None:
        tile.add_dep_helper(a.ins, b.ins, sync=False, reason=reason)


def build_kernel(wdotw: float):
    nc = bacc.Bacc("TRN2", target_bir_lowering=False, debug=False, num_devices=N_CORES)

    d_xt = nc.dram_tensor("xt", [384, NSB], F16, kind="ExternalInput")
    d_wq = nc.dram_tensor("wq", [384, INNER], F16, kind="ExternalInput")  # pre-scaled
    d_wk = nc.dram_tensor("wk", [DC, INNER], F16, kind="ExternalInput")
    d_wv = nc.dram_tensor("wv", [DC, INNER], F16, kind="ExternalInput")
    d_wo = nc.dram_tensor("wo", [INNER, DQ], F16, kind="ExternalInput")
    d_bo = nc.dram_tensor("bo", [384], F32, kind="ExternalInput")
    d_ctxkt = nc.dram_tensor("ctxkt", [DC, 5 * L], F16, kind="ExternalInput")
    d_ctxvt = nc.dram_tensor("ctxvt", [DC, 5 * L], F16, kind="ExternalInput")
    d_aet = nc.dram_tensor("aet", [H, L, NS], F16, kind="ExternalInput")
    d_yt = nc.dram_tensor("yt", [DQ, NSB], F16, kind="ExternalOutput")

    with tile.TileContext(nc) as tc:
        _emit(nc, tc, wdotw, d_xt, d_wq, d_wk, d_wv, d_wo, d_bo,
              d_ctxkt, d_ctxvt, d_aet, d_yt)
    nc.compile()
    return nc


def _emit(nc, tc, wdotw, d_xt, d_wq, d_wk, d_wv, d_wo, d_bo,
          d_ctxkt, d_ctxvt, d_aet, d_yt):
    from contextlib import ExitStack

    ctx = ExitStack()
    singles = ctx.enter_context(tc.tile_pool(name="singles", bufs=1))
    dram = ctx.enter_context(tc.tile_pool(name="dram", bufs=1, space="DRAM"))
    epool = ctx.enter_context(tc.tile_pool(name="epool", bufs=8))
    apool = ctx.enter_context(tc.tile_pool(name="apool", bufs=8))
    rzpool = ctx.enter_context(tc.tile_pool(name="rzpool", bufs=4))
    lnpool = ctx.enter_context(tc.tile_pool(name="lnpool", bufs=4))
    psim = ctx.enter_context(tc.tile_pool(name="psim", bufs=2, space="PSUM"))

    # ---- persistent SBUF tiles ----
    s_xt = singles.tile([128, 3, NSB], F16)
    s_wq = singles.tile([128, 3, INNER], F16)
    s_wk = singles.tile([128, 6, INNER], F16)
    s_wv = singles.tile([128, 6, INNER], F16)
    s_wo = singles.tile([128, 4, DQ], F16)
    s_bo = singles.tile([128, 3], F32)
    s_ctxkt = singles.tile([128, 6, 5 * L], F16)
    s_ctxvt = singles.tile([128, 6, 5 * L], F16)
    s_aet = singles.tile([L, H, NS], F16)
    s_qt = singles.tile([128, 4, NSB], F16)
    s_kt = singles.tile([128, 4, 5 * L], F16)
    s_ktc0 = singles.tile([128, 4, L], F16)
    s_vp = singles.tile([L, 5, INNER], F16)
    s_e0 = singles.tile([L, H, NS], F16)      # exp(branch-0 sims), pre-mask
    s_pvp = singles.tile([128, 4, NS], F32)   # cond PV partials (branches 1..3)
    s_om = singles.tile([128, 4, NSB], F16)   # merged outT (inner on partitions)
    s_y = singles.tile([128, 3, NSB], F16)
    s_lmax = singles.tile([L, H], F32)
    s_lm = singles.tile([L, 1], F32)
    s_maxrow8 = singles.tile([1, N_CORES * L], F32)
    s_wm = singles.tile([1, 1], F16)
    s_wmcol = singles.tile([L, 1], F32)
    ones_ext = singles.tile([L, 128], F16)
    ones_row = singles.tile([1, 128], F16)
    junk = singles.tile([128, INNER], F16)

    # ---- dummy collective: absorb the first-collective device barrier.
    # Reads an unwritten DRAM scratch tile so it has no data dependencies and
    # its trigger fires immediately at kernel start.
    din = dram.tile([1, 8], F32)
    dout = dram.tile([N_CORES, 8], F32)
    dummy_cc = nc.gpsimd.collective_compute(
        "AllGather", mybir.AluOpType.bypass,
        replica_groups=[list(range(N_CORES))],
        ins=[din.opt()], outs=[dout.opt()],
    )

    # ---- PE warm-up burst (keeps the HAM clock gate hot through the DMA) ----
    nc.vector.memset(junk[:], 0.03)
    pproj_cm = tc.tile_pool(name="pproj", bufs=2, space="PSUM")
    pproj = pproj_cm.__enter__()
    pjunk_cm = tc.tile_pool(name="pjunk", bufs=1, space="PSUM")
    pjunk = pjunk_cm.__enter__()
    pj = pjunk.tile([128, NS], F32)
    for _ in range(N_WARMUP_MM):
        nc.tensor.matmul(pj[:], junk[:, 0:128], junk[:, 0:NS], start=True, stop=True)

    # ---- phase A input DMA (lean: only what the collective path needs) ----
    nc.sync.dma_start(out=s_wq[:], in_=d_wq.ap().rearrange("(c p) f -> p c f", p=128))
    nc.sync.dma_start(out=s_xt[:, :, NS:NSB],
                      in_=d_xt.ap().rearrange("(c p) f -> p c f", p=128)[:, :, NS:NSB])
    nc.sync.dma_start(out=s_wk[:], in_=d_wk.ap().rearrange("(c p) f -> p c f", p=128))
    nc.sync.dma_start(out=s_ctxkt[:, :, L:2 * L],
                      in_=d_ctxkt.ap().rearrange("(c p) f -> p c f", p=128)[:, :, L:2 * L])

    nc.vector.memset(ones_ext[:], 1.0)
    nc.vector.memset(ones_row[:], 1.0)

    def qproj(half):
        for dc in range(4):
            p = pproj.tile([128, NS], F32, tag="proj")
            for kc in range(3):
                nc.tensor.matmul(
                    p[:],
                    s_wq[:, kc, dc * 128:(dc + 1) * 128],
                    s_xt[:, kc, half * NS:(half + 1) * NS],
                    start=(kc == 0), stop=(kc == 2),
                )
            nc.scalar.copy(s_qt[:, dc, half * NS:(half + 1) * NS], p[:])

    # ---- phase A: just enough for the branch-0 max -> collective ----
    qproj(1)                                     # cond-half qT
    for dc in range(4):                          # branch-0 kT slice
        p = pproj.tile([128, L], F32, tag="proj")
        for kc in range(6):
            nc.tensor.matmul(
                p[:],
                s_wk[:, kc, dc * 128:(dc + 1) * 128],
                s_ctxkt[:, kc, L:2 * L],
                start=(kc == 0), stop=(kc == 5),
            )
        nc.scalar.copy(s_ktc0[:, dc, :], p[:])

    def qk0(h, psum_slice):
        nc.tensor.matmul(
            psum_slice,
            s_ktc0[(h % 2) * 64:(h % 2) * 64 + 64, h // 2, :],
            s_qt[(h % 2) * 64:(h % 2) * 64 + 64, h // 2, NS:NSB],
            start=True, stop=True,
        )

    for hp in range(4):
        p = psim.tile([L, 2, NS], F32, tag="sim")
        qk0(2 * hp, p[:, 0, :])
        qk0(2 * hp + 1, p[:, 1, :])
        nc.vector.reduce_max(out=s_lmax[:, 2 * hp:2 * hp + 2], in_=p[:],
                             axis=AXX)
        nc.scalar.activation(s_e0[:, 2 * hp:2 * hp + 2, :], p[:], EXP)
    nc.vector.reduce_max(out=s_lm[:], in_=s_lmax[:], axis=AXX)
    nc.vector.tensor_scalar_mul(s_lm[:], s_lm[:], float(wdotw))

    cin = dram.tile([1, L], F32)
    cout = dram.tile([N_CORES, L], F32)
    nc.sync.dma_start(out=cin.rearrange("one f -> f one"), in_=s_lm[:])
    real_cc = nc.gpsimd.collective_compute(
        "AllGather", mybir.AluOpType.bypass,
        replica_groups=[list(range(N_CORES))],
        ins=[cin.opt()], outs=[cout.opt()],
    )
    _gate(real_cc, dummy_cc, "real collective after dummy")

    # ---- phase B input DMA ----
    nc.sync.dma_start(out=s_xt[:, :, 0:NS],
                      in_=d_xt.ap().rearrange("(c p) f -> p c f", p=128)[:, :, 0:NS])
    nc.sync.dma_start(out=s_wv[:], in_=d_wv.ap().rearrange("(c p) f -> p c f", p=128))
    nc.sync.dma_start(out=s_ctxvt[:], in_=d_ctxvt.ap().rearrange("(c p) f -> p c f", p=128))
    nc.sync.dma_start(out=s_wo[:], in_=d_wo.ap().rearrange("(c p) f -> p c f", p=128))
    nc.sync.dma_start(out=s_bo[:], in_=d_bo.ap().rearrange("(c p) -> p c", p=128))
    nc.sync.dma_start(out=s_ctxkt[:, :, 0:L],
                      in_=d_ctxkt.ap().rearrange("(c p) f -> p c f", p=128)[:, :, 0:L])
    nc.sync.dma_start(out=s_ctxkt[:, :, 2 * L:],
                      in_=d_ctxkt.ap().rearrange("(c p) f -> p c f", p=128)[:, :, 2 * L:])
    nc.sync.dma_start(out=s_aet[:], in_=d_aet.ap().rearrange("h p f -> p h f"))
    # tail input: gathered per-core branch-0 maxima (waits on the collective)
    nc.sync.dma_start(out=s_maxrow8[:], in_=cout.rearrange("r f -> (r f)"))

    pjunk_cm.__exit__(None, None, None)

    # ---- phase B: remaining projections ----
    qproj(0)                                     # uc-half qT
    for dc in range(4):                          # full kT (branch-0 cols unused)
        p = pproj.tile([128, 5 * L], F32, tag="proj")
        for kc in range(6):
            nc.tensor.matmul(
                p[:],
                s_wk[:, kc, dc * 128:(dc + 1) * 128],
                s_ctxkt[:, kc, :],
                start=(kc == 0), stop=(kc == 5),
            )
        nc.vector.tensor_copy(s_kt[:, dc, :], p[:])
    for g in range(5):                           # v, with 1/C folded into cond
        p = pproj.tile([128, INNER], F32, tag="proj")
        for kc in range(6):
            nc.tensor.matmul(
                p[0:L, :],
                s_ctxvt[:, kc, g * L:(g + 1) * L],
                s_wv[:, kc, :],
                start=(kc == 0), stop=(kc == 5),
            )
        if g == G_UC:
            nc.vector.tensor_copy(s_vp[:, g, :], p[0:L, :])
        else:
            nc.vector.tensor_scalar_mul(s_vp[:, g, :], p[0:L, :], 1.0 / C)
    pproj_cm.__exit__(None, None, None)

    pzb_cm = tc.tile_pool(name="pzb", bufs=1, space="PSUM")
    pzb = pzb_cm.__enter__()
    ppv_cm = tc.tile_pool(name="ppv", bufs=2, space="PSUM")
    ppv = ppv_cm.__enter__()

    def qk(g, h, psum_slice):
        cols = slice(0, NS) if g == G_UC else slice(NS, NSB)
        nc.tensor.matmul(
            psum_slice,
            s_kt[(h % 2) * 64:(h % 2) * 64 + 64, h // 2, g * L:(g + 1) * L],
            s_qt[(h % 2) * 64:(h % 2) * 64 + 64, h // 2, cols],
            start=True, stop=True,
        )

    def zmm(zb, e):
        # replicated per-column sums over the 77 key rows, on 128 partitions
        nc.tensor.matmul(zb[:, 0, :], ones_ext[:], e[:, 0, :], start=True, stop=True)
        nc.tensor.matmul(zb[:, 1, :], ones_ext[:], e[:, 1, :], start=True, stop=True)

    anchors = {}

    def normalize(e, zb, var):
        """Return attn tile a = e * (1/Z) [L,2,NS] f16 per the variant."""
        if var[0] == 'r':
            rz = rzpool.tile([128, 2, NS], F32, tag="rz")
            nc.vector.reciprocal_approx_fast(out=rz[:], in_=zb[:])
            rzs = rz[0:L, :, :]
        else:
            lnt = lnpool.tile([L, 2, NS], F32, tag="ln")
            nc.scalar.activation(lnt[:], zb[0:L, :, :], LN)
            rz = rzpool.tile([128, 2, NS], F16, tag="rzh")
            nc.scalar.activation(rz[0:L, :, :], lnt[:], EXP, scale=-1.0)
            rzs = rz[0:L, :, :]
        a = apool.tile([L, 2, NS], F16, tag="attn")
        eng = nc.vector if var[1] == 'd' else nc.gpsimd
        mi = eng.tensor_mul(a[:], e[:], rzs)
        if var[1] == 'g':
            _gate(mi, anchors.get("cc"), "gpsimd mul after collective trigger")
        return a

    def pv_head(pv, g, h, a_slice, start, stop):
        return nc.tensor.matmul(pv[:], s_vp[:, g, h * 64:(h + 1) * 64],
                                a_slice, start=start, stop=stop)

    def wo_oc(half, oc):
        ow = 128 if oc < 2 else 64
        p = ppv.tile([128, NS], F32, tag="pv")
        for kc in range(4):
            nc.tensor.matmul(
                p[0:ow, :],
                s_wo[:, kc, oc * 128:oc * 128 + ow],
                s_om[:, kc, half * NS:(half + 1) * NS],
                start=(kc == 0), stop=(kc == 3),
            )
        nc.scalar.add(s_y[0:ow, oc, half * NS:(half + 1) * NS], p[0:ow, :],
                      s_bo[0:ow, oc:oc + 1])

    def y_dma(half):
        for oc in range(3):
            ow = 128 if oc < 2 else 64
            nc.sync.dma_start(
                out=d_yt.ap()[oc * 128:oc * 128 + ow, half * NS:(half + 1) * NS],
                in_=s_y[0:ow, oc, half * NS:(half + 1) * NS])

    anchors["cc"] = real_cc

    # ---- phase B: uc group (QK, softmax via post-PV column scaling) ----
    for hp in range(4):
        p = psim.tile([L, 2, NS], F32, tag="sim")
        qk(0, 2 * hp, p[:, 0, :])
        qk(0, 2 * hp + 1, p[:, 1, :])
        e = epool.tile([L, 2, NS], F16, tag="e")
        nc.scalar.activation(e[:], p[:], EXP)
        zb = pzb.tile([128, 2, NS], F32, tag="zb")
        zmm(zb, e)
        rz = rzpool.tile([128, 2, NS], F32, tag="rz")
        nc.vector.reciprocal_approx_fast(out=rz[:], in_=zb[:])
        for k in range(2):
            h = 2 * hp + k
            pv = ppv.tile([64, NS], F32, tag="pv")
            pv_head(pv, 0, h, e[:, k, :], True, True)
            nc.vector.tensor_mul(s_om[(h % 2) * 64:(h % 2) * 64 + 64, hp, 0:NS],
                                 pv[:], rz[0:64, k, :])

    # ---- cond branches 1..3 (groups 2,3,4) pair-major, wo(0) interleaved ----
    for hp in range(4):
        atiles = {}
        for g in (2, 3, 4):
            p = psim.tile([L, 2, NS], F32, tag="sim")
            qk(g, 2 * hp, p[:, 0, :])
            qk(g, 2 * hp + 1, p[:, 1, :])
            e = epool.tile([L, 2, NS], F16, tag="e")
            anchors["act"] = nc.scalar.activation(e[:], p[:], EXP)
            zb = pzb.tile([128, 2, NS], F32, tag="zb")
            zmm(zb, e)
            atiles[g] = normalize(e, zb, COND_VAR[(hp, g)])
        for k in range(2):
            h = 2 * hp + k
            pv = ppv.tile([64, NS], F32, tag="pv")
            for i, g in enumerate((2, 3, 4)):
                anchors["mm"] = pv_head(pv, g, h, atiles[g][:, k, :],
                                        i == 0, i == 2)
            anchors["dve"] = nc.vector.tensor_copy(
                s_pvp[(h % 2) * 64:(h % 2) * 64 + 64, hp, :], pv[:])
        atiles.clear()
        if hp < 3:
            wo_oc(0, hp)    # uc output projection chunks fill PE gaps
    y_dma(0)

    # ---- tail: wmask, branch 0, PV merge, cond output projection ----
    red = nc.vector.reduce_max(out=s_wm[:], in_=s_maxrow8[:], axis=AXX)
    _gate(red, anchors.get("dve"), "defer wmask reduce behind phase B")
    p_wm = ppv.tile([128, NS], F32, tag="pv")
    bc = nc.tensor.matmul(p_wm[0:L, 0:1], ones_row[0:1, 0:L], s_wm[:],
                          start=True, stop=True)
    _gate(bc, anchors.get("mm"), "defer wmask bcast behind phase B matmuls")
    wmc = nc.scalar.copy(s_wmcol[:], p_wm[0:L, 0:1])
    _gate(wmc, anchors.get("act"), "defer wmask copy behind phase B exps")

    first = {}
    for hp in range(4):
        m = epool.tile([L, 2, NS], F16, tag="e")
        mex = nc.scalar.activation(m[:], s_aet[:, 2 * hp:2 * hp + 2, :], EXP,
                                   scale=s_wmcol[:])
        first.setdefault("act", mex)
        e0m = epool.tile([L, 2, NS], F16, tag="e")
        mm0 = nc.vector.tensor_mul(e0m[:], s_e0[:, 2 * hp:2 * hp + 2, :], m[:])
        first.setdefault("dve", mm0)
        zb = pzb.tile([128, 2, NS], F32, tag="zb")
        zmm(zb, e0m)
        a = normalize(e0m, zb, TAIL_VAR[hp])
        for k in range(2):
            h = 2 * hp + k
            pv = ppv.tile([64, NS], F32, tag="pv")
            pv_head(pv, 1, h, a[:, k, :], True, True)
            rows = slice((h % 2) * 64, (h % 2) * 64 + 64)
            nc.vector.tensor_tensor(out=s_om[rows, hp, NS:NSB], in0=pv[:],
                                    in1=s_pvp[rows, hp, :],
                                    op=mybir.AluOpType.add)
    _gate(first.get("act"), anchors.get("act"), "tail ACT after phase B ACT")
    _gate(first.get("dve"), anchors.get("dve"), "tail DVE after phase B DVE")

    # ---- cond half of the output projection ----
    for oc in range(3):
        wo_oc(1, oc)
    y_dma(1)

    ppv_cm.__exit__(None, None, None)
    pzb_cm.__exit__(None, None, None)
    ctx.pop_all().close()


_CACHE = {}


def kernel(x, uc_context, ck, cv, attn_extra, Wq, Wk, Wv, Wo, bo, t):
    global LAST_RESULTS
    x = np.ascontiguousarray(np.asarray(x, np.float32))
    uc_context = np.asarray(uc_context, np.float32)
    ck = np.asarray(ck, np.float32)
    cv = np.asarray(cv, np.float32)
    attn_extra = np.asarray(attn_extra, np.float32)
    Wq = np.asarray(Wq, np.float32)
    Wk = np.asarray(Wk, np.float32)
    Wv = np.asarray(Wv, np.float32)
    Wo = np.asarray(Wo, np.float32)
    bo = np.asarray(bo, np.float32)
    tv = float(np.asarray(t))
    wdotw = W_DOT * (tv / TOTAL_STEP) * SCHED

    if wdotw not in _CACHE:
        _CACHE[wdotw] = build_kernel(wdotw)
    nc = _CACHE[wdotw]

    # host-side input prep (layout only)
    wq_pad = np.zeros((384, INNER), np.float16)
    wq_pad[:DQ] = (Wq * SCALE).astype(np.float16)
    bo_pad = np.zeros((384,), np.float32)
    bo_pad[:DQ] = bo
    wk16 = Wk.astype(np.float16)
    wv16 = Wv.astype(np.float16)
    wo16 = Wo.astype(np.float16)
    ctxK = np.concatenate([uc_context[0][None], ck[:, 0]], axis=0)  # [5, 77, 768]
    ctxV = np.concatenate([uc_context[0][None], cv[:, 0]], axis=0)
    ctxkt = np.ascontiguousarray(ctxK.transpose(2, 0, 1).reshape(DC, 5 * L)).astype(np.float16)
    ctxvt = np.ascontiguousarray(ctxV.transpose(2, 0, 1).reshape(DC, 5 * L)).astype(np.float16)

    in_maps = []
    for c in range(N_CORES):
        rows = slice(c * NS, (c + 1) * NS)
        xt = np.zeros((384, NSB), np.float16)
        xt[:DQ, :NS] = x[0, rows].T.astype(np.float16)
        xt[:DQ, NS:] = x[1, rows].T.astype(np.float16)
        aet = np.ascontiguousarray(
            attn_extra[:, rows, :].transpose(0, 2, 1)).astype(np.float16)
        in_maps.append({
            "xt": xt, "wq": wq_pad, "wk": wk16, "wv": wv16, "wo": wo16, "bo": bo_pad,
            "ctxkt": ctxkt, "ctxvt": ctxvt, "aet": aet,
        })

    import os as _os
    _tc = None
    if _os.environ.get("KERNEL_TRACE_ALL") == "1":
        _tc = list(range(N_CORES))
    res = bass_utils.run_bass_kernel_spmd(
        nc, in_maps, core_ids=list(range(N_CORES)), trace=TRACE, trace_cores=_tc,
    )
    LAST_RESULTS = res

    out = np.empty((2, N, DQ), np.float32)
    for c in range(N_CORES):
        rows = slice(c * NS, (c + 1) * NS)
        yt = res.results[c]["yt"]
        out[0, rows] = yt[:, :NS].T.astype(np.float32)
        out[1, rows] = yt[:, NS:].T.astype(np.float32)
    return out
